# revision 6
# baseline (speedup 1.0000x reference)
"""ALIGNN forward pass on 8 trn2 NeuronCores (Bass/Tile).

Sharding: edges sorted by dst with node-aligned cuts; triplets sorted by
(renumbered) t_dst with edge-aligned cuts. Device d owns a node range, an
edge range and a triplet range. x replicated (block AllGather per layer);
y sharded (gather table allgathered row-major per line layer); z local.

Message pass per egcn: src side via one random row gather of a pre-fused
[A.s || B.s + b] row-major table (dma_gather for the 8192-row node table,
indirect_dma_start for the allgathered 8*E_CAP-row edge table); dst side and
segment_sum via one-hot window matmuls on PE (dst-sorted => dense windows);
BatchNorm via masked matmul stats + one batched AllReduce per layer.
"""
import contextlib
import ctypes
import os
import sys
import types

import numpy as np

# ----------------------------------------------------------- env shims
# (kernel.py must be self-contained: these patch the local runtime only)


def _install_profile_shim():
    try:
        if 'antenv.axon_hooks' not in sys.modules:
            mod = types.ModuleType('antenv.axon_hooks')
            _hook = [None]
            mod.set_axon_ntff_profile_hook = lambda h: _hook.__setitem__(0, h)
            mod.get_axon_ntff_profile_hook = lambda: _hook[0]
            sys.modules['antenv.axon_hooks'] = mod
            import antenv
            antenv.axon_hooks = mod
        from antenv.axon_hooks import set_axon_ntff_profile_hook
        lib = ctypes.CDLL('/opt/axon/libaxon_pjrt.so')
        if not hasattr(lib, 'axon_start_nrt_profile'):
            return
        lib.axon_start_nrt_profile.argtypes = [
            ctypes.POINTER(ctypes.c_int64), ctypes.c_size_t]
        lib.axon_start_nrt_profile.restype = ctypes.c_int64
        lib.axon_stop_nrt_profile.argtypes = [ctypes.c_char_p]
        lib.axon_stop_nrt_profile.restype = ctypes.c_int64
        lib.axon_reset.restype = ctypes.c_int64

        @contextlib.contextmanager
        def robust_hook(output_dir, device_ids):
            import jax
            jax.devices()

            def _start():
                if device_ids:
                    ids = (ctypes.c_int64 * len(device_ids))(*device_ids)
                    return lib.axon_start_nrt_profile(ids, len(device_ids))
                return lib.axon_start_nrt_profile(None, 0)

            rc = _start()
            if rc != 0:
                lib.axon_reset()
                rc = _start()
            if rc != 0:
                raise RuntimeError(f"axon_start_nrt_profile rc={rc}")
            try:
                yield
            finally:
                n = lib.axon_stop_nrt_profile(str(output_dir).encode())
                if n < 0:
                    lib.axon_reset()
                    raise RuntimeError(f"axon_stop_nrt_profile rc={n}")

        set_axon_ntff_profile_hook(robust_hook)
        from concourse import bass_utils
        bass_utils.upload_artifacts = lambda tmpdir: f"local:{tmpdir}"
    except Exception:
        pass


def _install_drain_patch():
    # this environment's walrus rejects >1 sync wait per instruction; split
    # the kernel-tail drain's waits across one nop per logical proc.
    from concourse import tile as _tile
    from concourse.vector_clock import ScopedClock

    def _patched(self, tick_clock, wait_clock):
        vc = tick_clock.global_clock
        for proc in range(len(vc)):
            t = vc[proc]
            if t > 0:
                nop = self.nc.sync.nop(nofuse=True, hint=f"drain_split_{proc}")
                pc = ScopedClock()
                pc.require_at_least(None, proc, t)
                wait_clock.add_sem_waits(nop.ins, pc)
        drain_inst = self.nc.sync.drain()
        wait_clock.add_sem_waits(
            drain_inst.ins, ScopedClock({None: tick_clock.global_clock}))
        si = drain_inst.ins.sync_info
        if si is not None and len(si.on_wait) > 1:
            si.on_wait = [si.on_wait[0]]
        self.nc.all_engine_barrier()
        assert self.sems is not None
        popped = self.nc._tile_sem_poison_stack.pop()
        assert popped is self._sem_poison
        self.nc.clear_and_free_semaphores(list(self.sems.allocated().values()))
        self.nc.all_engine_barrier()

    _tile.TileContext._drain_and_barrier = _patched


_install_profile_shim()
_install_drain_patch()

from concourse import bass, bacc, mybir, tile
from concourse.bass_utils import run_bass_kernel_spmd
from concourse.masks import make_identity

P = 128
H = 128
NC = 8
F32 = mybir.dt.float32
EPS_BN = 1e-5
AF = mybir.ActivationFunctionType
ALU = mybir.AluOpType


def _ceil_to(x, m):
    return ((x + m - 1) // m) * m


class _O:
    pass


# ------------------------------------------------------------------ host plan

def build_plan(atom_features, bondlength, angle_features, params,
               edge_src, edge_dst, t_src, t_dst, node2graph):
    pl = _O()
    N = atom_features.shape[0]
    E = edge_src.shape[0]
    T = t_src.shape[0]
    G = int(node2graph.max()) + 1
    pl.N, pl.E, pl.T, pl.G = N, E, T, G
    EB, AB = 80, 40
    pl.EB, pl.AB = EB, AB

    e_perm = np.argsort(edge_dst, kind="stable")
    sdst = edge_dst[e_perm]
    ssrc = edge_src[e_perm]
    cut_e = np.zeros(NC + 1, dtype=np.int64)
    for d in range(1, NC):
        c = d * E // NC
        while 0 < c < E and sdst[c] == sdst[c - 1]:
            c += 1
        cut_e[d] = c
    cut_e[NC] = E
    node_cut = np.zeros(NC + 1, dtype=np.int64)
    for d in range(1, NC):
        node_cut[d] = sdst[cut_e[d]] if cut_e[d] < E else N
    node_cut[NC] = N
    E_d = np.diff(cut_e)
    W_d = np.diff(node_cut)
    pl.cut_e, pl.node_cut, pl.E_d, pl.W_d = cut_e, node_cut, E_d, W_d
    pl.E_CAP = _ceil_to(int(E_d.max()), 1024)
    pl.W_CAP = _ceil_to(int(W_d.max()) + 1, 512)

    inv_e = np.empty(E, dtype=np.int64)
    inv_e[e_perm] = np.arange(E)
    ts_new = inv_e[t_src]
    td_new = inv_e[t_dst]
    t_perm = np.argsort(td_new, kind="stable")
    sts = ts_new[t_perm]
    std = td_new[t_perm]
    cut_t = np.searchsorted(std, cut_e).astype(np.int64)
    cut_t[0], cut_t[NC] = 0, T
    T_d = np.diff(cut_t)
    pl.cut_t, pl.T_d = cut_t, T_d
    pl.T_CAP = _ceil_to(int(T_d.max()), 1024)

    ECAP, TCAP, WCAP = pl.E_CAP, pl.T_CAP, pl.W_CAP
    n_et, n_tt = ECAP // P, TCAP // P
    pl.n_et, pl.n_tt = n_et, n_tt
    pl.n_wb, pl.n_eb = WCAP // P, ECAP // P
    pl.n_wwin, pl.n_ewin = WCAP // 512, ECAP // 512

    e_blocks = [set() for _ in range(n_et)]
    e_scat = [[n_et, -1] for _ in range(pl.n_wwin)]
    t_blocks = [set() for _ in range(n_tt)]
    t_scat = [[n_tt, -1] for _ in range(pl.n_ewin)]
    pl.dev = []
    for d in range(NC):
        dv = _O()
        el, eh = cut_e[d], cut_e[d + 1]
        ne = int(eh - el)
        src_g = np.full(ECAP, ssrc[eh - 1] if ne else 0, dtype=np.int64)
        src_g[:ne] = ssrc[el:eh]
        dst_l = np.full(ECAP, 2.0 * WCAP, dtype=np.float32)
        dst_l[:ne] = (sdst[el:eh] - node_cut[d]).astype(np.float32)
        emask = np.zeros(ECAP, dtype=np.float32)
        emask[:ne] = 1.0
        dv.src_idx16 = np.tile(
            src_g.astype(np.int16).reshape(ECAP // 16, 16).T, (8, 1)).copy()
        dv.dst_l_col = dst_l.reshape(n_et, P).T.copy()
        dv.emask_col = emask.reshape(n_et, P).T.copy()
        dv.dst_row_bc = np.tile(dst_l[None, :], (P, 1))
        dv.emask_row_bc = np.tile(emask[None, :], (P, 1))
        for j in range(n_et):
            dd = dst_l[j * P:(j + 1) * P]
            rl = dd[dd < WCAP]
            if rl.size:
                lo, hi = int(rl.min()), int(rl.max())
                for b in range(lo // P, hi // P + 1):
                    e_blocks[j].add(b)
                for k in range(lo // 512, hi // 512 + 1):
                    e_scat[k][0] = min(e_scat[k][0], j)
                    e_scat[k][1] = max(e_scat[k][1], j)

        tl, th = cut_t[d], cut_t[d + 1]
        nt = int(th - tl)
        ts_l = np.full(TCAP, sts[th - 1] if nt else 0, dtype=np.int64)
        ts_l[:nt] = sts[tl:th]
        tdl = np.full(TCAP, 2.0 * ECAP, dtype=np.float32)
        tdl[:nt] = (std[tl:th] - cut_e[d]).astype(np.float32)
        tmask = np.zeros(TCAP, dtype=np.float32)
        tmask[:nt] = 1.0
        owner = np.clip(np.searchsorted(cut_e, ts_l, side="right") - 1, 0, NC - 1)
        gidx = (owner * ECAP + (ts_l - cut_e[owner])).astype(np.int32)
        dv.ts_idx32 = gidx.reshape(n_tt, P).T.copy()
        dv.tdst_l_col = tdl.reshape(n_tt, P).T.copy()
        dv.tmask_col = tmask.reshape(n_tt, P).T.copy()
        dv.tdst_row_bc = np.tile(tdl[None, :], (P, 1))
        dv.tmask_row_bc = np.tile(tmask[None, :], (P, 1))
        for j in range(n_tt):
            dd = tdl[j * P:(j + 1) * P]
            rl = dd[dd < ECAP]
            if rl.size:
                lo, hi = int(rl.min()), int(rl.max())
                for b in range(lo // P, hi // P + 1):
                    t_blocks[j].add(b)
                for k in range(lo // 512, hi // 512 + 1):
                    t_scat[k][0] = min(t_scat[k][0], j)
                    t_scat[k][1] = max(t_scat[k][1], j)

        bl = np.zeros(ECAP, dtype=np.float32)
        bl[:ne] = bondlength[e_perm[el:eh]]
        dv.bond_bc = np.tile(bl[None, :], (EB, 1))
        af = np.zeros(TCAP, dtype=np.float32)
        af[:nt] = angle_features[t_perm[tl:th]]
        dv.angle_bc = np.tile(af[None, :], (AB, 1))
        nm = np.zeros(WCAP, dtype=np.float32)
        nm[:W_d[d]] = 1.0
        dv.nmask_bc = np.tile(nm[None, :], (P, 1))
        dv.ncut_bc = np.tile(
            (np.arange(WCAP, dtype=np.float32) + node_cut[d])[None, :], (P, 1))
        pl.dev.append(dv)

    pl.e_blocks = [sorted(b) for b in e_blocks]
    pl.e_scat = e_scat
    pl.t_blocks = [sorted(b) for b in t_blocks]
    pl.t_scat = t_scat

    pl.afT = np.ascontiguousarray(atom_features.T.astype(np.float32))
    counts = np.bincount(node2graph, minlength=G).astype(np.float32)
    Gm = np.zeros((N, G), dtype=np.float32)
    Gm[np.arange(N), node2graph] = 1.0 / np.maximum(counts[node2graph], 1.0)
    pl.G_mat = Gm
    pl.iota512_bc = np.tile(np.arange(512, dtype=np.float32)[None, :], (P, 1))
    pl.iotaP_col = np.arange(P, dtype=np.float32).reshape(P, 1)
    pl.iotaN_col = np.arange(N, dtype=np.float32).reshape(N // P, P).T.copy()

    pr = {}

    def addlin(pref, p):
        pr[pref + "W"] = np.asarray(p["W"], np.float32)
        pr[pref + "b"] = np.asarray(p["b"], np.float32).reshape(-1, 1)

    def addmlp(pref, p):
        addlin(pref, p)
        pr[pref + "g"] = np.asarray(p["gamma"], np.float32).reshape(-1, 1)
        pr[pref + "be"] = np.asarray(p["beta"], np.float32).reshape(-1, 1)

    addmlp("atom_", params["atom_emb"])
    addmlp("eemb0_", params["edge_emb"][0])
    addmlp("eemb1_", params["edge_emb"][1])
    addmlp("aemb0_", params["angle_emb"][0])
    addmlp("aemb1_", params["angle_emb"][1])

    def addegcn(pref, p):
        addlin(pref + "sg_", p["src_gate"])
        addlin(pref + "dg_", p["dst_gate"])
        addlin(pref + "eg_", p["edge_gate"])
        addlin(pref + "su_", p["src_update"])
        addlin(pref + "du_", p["dst_update"])
        for nm_, k in [("bnn_g", "bn_nodes_g"), ("bnn_b", "bn_nodes_b"),
                       ("bne_g", "bn_edges_g"), ("bne_b", "bn_edges_b")]:
            pr[pref + nm_] = np.asarray(p[k], np.float32).reshape(-1, 1)

    pl.layer_prefixes = []
    lix = 0
    for lp in params["alignn"]:
        addegcn(f"l{lix}_", lp["node"]); pl.layer_prefixes.append((f"l{lix}_", "node")); lix += 1
        addegcn(f"l{lix}_", lp["edge"]); pl.layer_prefixes.append((f"l{lix}_", "line")); lix += 1
    for p in params["gcn"]:
        addegcn(f"l{lix}_", p); pl.layer_prefixes.append((f"l{lix}_", "node")); lix += 1
    addlin("fc_", params["fc"])
    pr["fc_b"] = np.tile(np.asarray(params["fc"]["b"], np.float32).reshape(1, 1), (P, 1))
    for pref, _ in pl.layer_prefixes:
        bm = (pr[pref + "sg_b"] + pr[pref + "dg_b"] + pr[pref + "eg_b"]).reshape(-1)
        pr[pref + "bm_bc"] = np.tile(bm[None, :], (P, 1)).astype(np.float32)
    pl.params = pr

    pl.e_centers = np.linspace(0.0, 8.0, EB, dtype=np.float32).reshape(-1, 1)
    pl.e_gamma = float(1.0 / (8.0 / (EB - 1)))
    pl.a_centers = np.linspace(-1.0, 1.0, AB, dtype=np.float32).reshape(-1, 1)
    pl.a_gamma = float(1.0 / (2.0 / (AB - 1)))
    return pl


# ------------------------------------------------------------------ program

def build_program(pl):
    nc = bacc.Bacc("TRN2")
    N, E, T, G = pl.N, pl.E, pl.T, pl.G
    EB, AB = pl.EB, pl.AB
    ECAP, TCAP, WCAP = pl.E_CAP, pl.T_CAP, pl.W_CAP
    CORE_IDS = list(range(NC))

    def inp(name, shape, dt=F32):
        return nc.declare_dram_parameter(name, list(shape), dt, isOutput=False)

    src_idx16 = inp("src_idx16", [P, ECAP // 16], mybir.dt.int16)
    dst_l_col = inp("dst_l_col", [P, pl.n_et])
    emask_col = inp("emask_col", [P, pl.n_et])
    dst_row_bc = inp("dst_row_bc", [P, ECAP])
    emask_row_bc = inp("emask_row_bc", [P, ECAP])
    ts_idx32 = inp("ts_idx32", [P, pl.n_tt], mybir.dt.int32)
    tdst_l_col = inp("tdst_l_col", [P, pl.n_tt])
    tmask_col = inp("tmask_col", [P, pl.n_tt])
    tdst_row_bc = inp("tdst_row_bc", [P, TCAP])
    tmask_row_bc = inp("tmask_row_bc", [P, TCAP])
    bond_bc = inp("bond_bc", [EB, ECAP])
    angle_bc = inp("angle_bc", [AB, TCAP])
    nmask_bc = inp("nmask_bc", [P, WCAP])
    ncut_bc = inp("ncut_bc", [P, WCAP])
    afT_i = inp("afT", [pl.afT.shape[0], N])
    G_mat = inp("G_mat", [N, G])
    iota512_i = inp("iota512_bc", [P, 512])
    iotaP_i = inp("iotaP_col", [P, 1])
    iotaN_i = inp("iotaN_col", [P, N // P])
    e_cent = inp("e_centers", [EB, 1])
    a_cent = inp("a_centers", [AB, 1])
    prm = {k: inp("p_" + k, v.shape) for k, v in pl.params.items()}
    out_p = nc.declare_dram_parameter("out", [G, 1], F32, isOutput=True)

    dT1x = nc.dram_tensor("dT1x", [N, 2 * H], F32)
    dT2y = nc.dram_tensor("dT2y", [ECAP, H], F32)
    dT1y_in = nc.dram_tensor("dT1y_in", [ECAP, 2 * H], F32)
    dT1y = nc.dram_tensor("dT1y", [NC * ECAP, 2 * H], F32, addr_space="Shared")
    dyT = nc.dram_tensor("dyT", [P, ECAP], F32)
    dzT = nc.dram_tensor("dzT", [P, TCAP], F32)
    dmT = nc.dram_tensor("dmT", [P, TCAP], F32)
    dvb = nc.dram_tensor("dvb", [P, max(WCAP, ECAP)], F32)
    dv0e = nc.dram_tensor("dv0e", [64, ECAP], F32)
    dv0a = nc.dram_tensor("dv0a", [64, TCAP], F32)
    dxag_in = nc.dram_tensor("dxag_in", [P, WCAP], F32)
    dxag = nc.dram_tensor("dxag", [NC * P, WCAP], F32, addr_space="Shared")
    dar_in = nc.dram_tensor("dar_in", [P, 4], F32)
    dar = nc.dram_tensor("dar", [P, 4], F32, addr_space="Shared")

    MM = dict(skip_group_check=True)

    from contextlib import ExitStack
    _stk = ExitStack()
    with tile.TileContext(nc) as tc:
        pool = _stk.enter_context(tc.tile_pool(name="main", bufs=1))
        wp = _stk.enter_context(tc.tile_pool(name="work", bufs=2))
        pp = _stk.enter_context(tc.tile_pool(name="psA", bufs=2, space="PSUM"))
        ppw = _stk.enter_context(tc.tile_pool(name="psW", bufs=1, space="PSUM"))

        ident = pool.tile([P, P], F32, tag="ident")
        make_identity(nc, ident[:])
        xT = pool.tile([P, N], F32, tag="xT")
        xT_own = pool.tile([P, WCAP], F32, tag="xT_own")
        T2x_strip = pool.tile([P, pl.n_wb * P], F32, tag="T2x")
        iota_t = pool.tile([P, 512], F32, tag="iota")
        nc.sync.dma_start(out=iota_t[:], in_=iota512_i[:])
        iotaP = pool.tile([P, 1], F32, tag="iotaP")
        nc.sync.dma_start(out=iotaP[:], in_=iotaP_i[:])
        iotaN = pool.tile([P, N // P], F32, tag="iotaN")
        nc.sync.dma_start(out=iotaN[:], in_=iotaN_i[:])

        params_sb = {}

        def W(name):
            if name not in params_sb:
                t = pool.tile(list(pl.params[name].shape), F32, tag="p_" + name)
                nc.sync.dma_start(out=t[:], in_=prm[name][:])
                params_sb[name] = t
            return params_sb[name]

        mu = pool.tile([P, 1], F32, tag="mu")
        rs = pool.tile([P, 1], F32, tag="rs")
        sc = pool.tile([P, 1], F32, tag="sc")
        bi = pool.tile([P, 1], F32, tag="bi")
        tmp1 = pool.tile([P, 1], F32, tag="tmp1")

        def finish_bn(stat_ap, count, gamma, beta, Q=P):
            nc.vector.tensor_scalar_mul(mu[:Q], stat_ap[:Q, 0:1], 1.0 / count)
            nc.vector.tensor_scalar_mul(rs[:Q], stat_ap[:Q, 1:2], 1.0 / count)
            nc.vector.tensor_mul(tmp1[:Q], mu[:Q], mu[:Q])
            nc.vector.tensor_sub(rs[:Q], rs[:Q], tmp1[:Q])
            nc.vector.tensor_scalar_add(rs[:Q], rs[:Q], EPS_BN)
            nc.scalar.activation(rs[:Q], rs[:Q], AF.Sqrt)
            nc.vector.reciprocal(rs[:Q], rs[:Q])
            nc.vector.tensor_mul(sc[:Q], gamma, rs[:Q])
            nc.vector.tensor_mul(bi[:Q], mu[:Q], sc[:Q])
            nc.vector.tensor_sub(bi[:Q], beta, bi[:Q])

        ar_in_sb = pool.tile([P, 4], F32, tag="ar_in")
        ar_out_sb = pool.tile([P, 4], F32, tag="ar_out")

        def do_ar(stats_list):
            nc.vector.memset(ar_in_sb[:], 0.0)
            for ap_, Q, co in stats_list:
                nc.vector.tensor_copy(ar_in_sb[:Q, co:co + 2], ap_[:Q, :])
            nc.sync.dma_start(out=dar_in[:], in_=ar_in_sb[:])
            nc.gpsimd.collective_compute(
                "AllReduce", ALU.add,
                ins=[dar_in[:]], outs=[dar[:]], replica_groups=[CORE_IDS])
            nc.sync.dma_start(out=ar_out_sb[:], in_=dar[:])

        stat_a1 = pool.tile([P, 2], F32, tag="stat_a1")
        stat_a2 = pool.tile([P, 2], F32, tag="stat_a2")

        def accum_stats(stat, v_ap, mask_ap, Q, first):
            # stat[:,0]+=sum(v*mask) ; stat[:,1]+=sum(v^2*mask)  (free-dim reduce)
            t = wp.tile([P, 512], F32, tag="ms_t")
            r = wp.tile([P, 2], F32, tag="ms_r")
            nc.vector.tensor_mul(t[:Q], v_ap, mask_ap)
            nc.vector.reduce_sum(r[:Q, 0:1], t[:Q], axis=mybir.AxisListType.X)
            nc.vector.tensor_mul(t[:Q], t[:Q], v_ap)
            nc.vector.reduce_sum(r[:Q, 1:2], t[:Q], axis=mybir.AxisListType.X)
            if first:
                nc.vector.tensor_copy(stat[:Q, :], r[:Q, :])
            else:
                nc.vector.tensor_add(stat[:Q, :], stat[:Q, :], r[:Q, :])

        # ====================================================== atom embedding
        AFP = pl.afT.shape[0]
        for c in range(N // 512):
            afc = wp.tile([AFP, 512], F32, tag="den")
            nc.sync.dma_start(out=afc[:], in_=afT_i[:, c * 512:(c + 1) * 512])
            pm = pp.tile([P, 512], F32, tag="ps")
            nc.tensor.matmul(out=pm[:], lhsT=W("atom_W")[:],
                             rhs=afc[:], start=True, stop=True, **MM)
            nc.vector.tensor_scalar_add(xT[:, c * 512:(c + 1) * 512], pm[:],
                                        W("atom_b")[:])
        ones_mask = pool.tile([P, 512], F32, tag="ones")
        nc.vector.memset(ones_mask[:], 1.0)
        for c in range(N // 512):
            accum_stats(stat_a1, xT[:, c * 512:(c + 1) * 512], ones_mask[:], P, c == 0)
        finish_bn(stat_a1, N, W("atom_g")[:], W("atom_be")[:])
        for c in range(N // 512):
            nc.scalar.activation(xT[:, c * 512:(c + 1) * 512],
                                 xT[:, c * 512:(c + 1) * 512], AF.Silu,
                                 bias=bi[:], scale=sc[:])

        # =============================================== edge/angle embeddings
        def emb_stage0(src_i, cent_i, gmm, Q_in, CAP, w0, b0, dst_d, mask_i, stat):
            cent_sb = wp.tile([Q_in, 1], F32, tag="dcol")
            nc.sync.dma_start(out=cent_sb[:], in_=cent_i[:])
            for c in range(CAP // 512):
                sl_i = wp.tile([Q_in, 512], F32, tag="den")
                nc.sync.dma_start(out=sl_i[:], in_=src_i[:, c * 512:(c + 1) * 512])
                t = wp.tile([Q_in, 512], F32, tag="hsl")
                nc.vector.tensor_scalar(t[:], sl_i[:], cent_sb[:], None, ALU.subtract)
                nc.vector.tensor_mul(t[:], t[:], t[:])
                nc.scalar.activation(t[:], t[:], AF.Exp, scale=-gmm)
                pm = pp.tile([64, 512], F32, tag="ps")
                nc.tensor.matmul(out=pm[:], lhsT=w0[:], rhs=t[:], start=True, stop=True, **MM)
                v = wp.tile([64, 512], F32, tag="vup")
                nc.vector.tensor_scalar_add(v[:], pm[:], b0[:])
                msk = wp.tile([P, 512], F32, tag="yupd")
                nc.sync.dma_start(out=msk[:64], in_=mask_i[:64, c * 512:(c + 1) * 512])
                accum_stats(stat, v[:], msk[:64], 64, c == 0)
                nc.sync.dma_start(out=dst_d[:, c * 512:(c + 1) * 512], in_=v[:])

        emb_stage0(bond_bc, e_cent, pl.e_gamma, EB, ECAP, W("eemb0_W"),
                   W("eemb0_b"), dv0e, emask_row_bc, stat_a1)
        emb_stage0(angle_bc, a_cent, pl.a_gamma, AB, TCAP, W("aemb0_W"),
                   W("aemb0_b"), dv0a, tmask_row_bc, stat_a2)
        do_ar([(stat_a1, 64, 0), (stat_a2, 64, 2)])
        nc.vector.tensor_copy(stat_a1[:64], ar_out_sb[:64, 0:2])
        nc.vector.tensor_copy(stat_a2[:64], ar_out_sb[:64, 2:4])

        def emb_stage1(src_d, CAP, stat, count, g0, be0, w1, b1, dst_d, mask_i, stat2):
            finish_bn(stat, count, g0, be0, Q=64)
            for c in range(CAP // 512):
                v = wp.tile([64, 512], F32, tag="vup")
                nc.sync.dma_start(out=v[:], in_=src_d[:, c * 512:(c + 1) * 512])
                nc.scalar.activation(v[:], v[:], AF.Silu, bias=bi[:64], scale=sc[:64])
                pm = pp.tile([P, 512], F32, tag="ps")
                nc.tensor.matmul(out=pm[:], lhsT=w1[:], rhs=v[:], start=True, stop=True, **MM)
                v2 = wp.tile([P, 512], F32, tag="vmask")
                nc.vector.tensor_scalar_add(v2[:], pm[:], b1[:])
                msk = wp.tile([P, 512], F32, tag="yupd")
                nc.sync.dma_start(out=msk[:], in_=mask_i[:, c * 512:(c + 1) * 512])
                accum_stats(stat2, v2[:], msk[:], P, c == 0)
                nc.sync.dma_start(out=dst_d[:, c * 512:(c + 1) * 512], in_=v2[:])

        emb_stage1(dv0e, ECAP, stat_a1, E, W("eemb0_g")[:64], W("eemb0_be")[:64],
                   W("eemb1_W"), W("eemb1_b"), dyT, emask_row_bc, stat_a1)
        emb_stage1(dv0a, TCAP, stat_a2, T, W("aemb0_g")[:64], W("aemb0_be")[:64],
                   W("aemb1_W"), W("aemb1_b"), dzT, tmask_row_bc, stat_a2)
        do_ar([(stat_a1, P, 0), (stat_a2, P, 2)])
        nc.vector.tensor_copy(stat_a1[:], ar_out_sb[:, 0:2])
        nc.vector.tensor_copy(stat_a2[:], ar_out_sb[:, 2:4])

        def emb_finish(dst_d, CAP, stat, count, g1, be1):
            finish_bn(stat, count, g1, be1)
            for c in range(CAP // 512):
                v = wp.tile([P, 512], F32, tag="vmask")
                nc.sync.dma_start(out=v[:], in_=dst_d[:, c * 512:(c + 1) * 512])
                nc.scalar.activation(v[:], v[:], AF.Silu, bias=bi[:], scale=sc[:])
                nc.sync.dma_start(out=dst_d[:, c * 512:(c + 1) * 512], in_=v[:])

        emb_finish(dyT, ECAP, stat_a1, E, W("eemb1_g"), W("eemb1_be"))
        emb_finish(dzT, TCAP, stat_a2, T, W("aemb1_g"), W("aemb1_be"))

        # ------------------------------------------------ xT_own via one-hot
        def extract_own():
            for k in range(WCAP // 512):
                pxo = ppw.tile([P, 512], F32, tag="pwh0")
                ncut_c = wp.tile([P, 512], F32, tag="hsl")
                nc.sync.dma_start(out=ncut_c[:], in_=ncut_bc[:, k * 512:(k + 1) * 512])
                for cblk in range(N // P):
                    selc = wp.tile([P, 512], F32, tag="s_ew")
                    nc.vector.tensor_tensor(
                        out=selc[:], in0=iotaN[:, cblk:cblk + 1].to_broadcast([P, 512]),
                        in1=ncut_c[:], op=ALU.is_equal)
                    ptx = pp.tile([P, P], F32, tag="ps")
                    nc.tensor.transpose(out=ptx[:], in_=xT[:, cblk * P:(cblk + 1) * P],
                                        identity=ident[:])
                    xrow = wp.tile([P, P], F32, tag="xrow")
                    nc.scalar.activation(xrow[:], ptx[:], AF.Copy)
                    nc.tensor.matmul(out=pxo[:], lhsT=xrow[:], rhs=selc[:],
                                     start=(cblk == 0), stop=(cblk == N // P - 1), **MM)
                nc.scalar.activation(xT_own[:, k * 512:(k + 1) * 512], pxo[:], AF.Copy)

        extract_own()

        # ============================================================ layers
        def transpose_block(dst_dram_rows, src_fm_ap, bias=None):
            """[f,128] feature-major SBUF slice -> row-major [128,f] DRAM rows."""
            pt = pp.tile([P, P], F32, tag="ps")
            nc.tensor.transpose(out=pt[:], in_=src_fm_ap, identity=ident[:])
            t2 = wp.tile([P, P], F32, tag="tb_o")
            nc.scalar.activation(t2[:], pt[:], AF.Copy)
            nc.sync.dma_start(out=dst_dram_rows, in_=t2[:])

        def egcn_layer(pref, kind):
            if kind == "node":
                n_tiles, CAP, WIN = pl.n_et, ECAP, WCAP
                blocks, scat, n_win = pl.e_blocks, pl.e_scat, pl.n_wwin
                dstrow_i, dstcol_i, mcol_i = dst_row_bc, dst_l_col, emask_col
                count_e, count_n = E, N
                edge_d = dyT
            else:
                n_tiles, CAP, WIN = pl.n_tt, TCAP, ECAP
                blocks, scat, n_win = pl.t_blocks, pl.t_scat, pl.n_ewin
                dstrow_i, dstcol_i, mcol_i = tdst_row_bc, tdst_l_col, tmask_col
                count_e, count_n = T, E
                edge_d = dzT
            bm_bc = W(pref + "bm_bc")

            # ---- tables
            if kind == "node":
                for blk in range(N // P):
                    for half, (wn, bn_) in enumerate([("sg_", None), ("du_", "du_b")]):
                        pm = pp.tile([P, P], F32, tag="ps")
                        nc.tensor.matmul(out=pm[:], lhsT=W(pref + wn + "W")[:],
                                         rhs=xT[:, blk * P:(blk + 1) * P],
                                         start=True, stop=True, **MM)
                        t = wp.tile([P, P], F32, tag="tb_i")
                        if bn_ is None:
                            nc.scalar.activation(t[:], pm[:], AF.Copy)
                        else:
                            nc.vector.tensor_scalar_add(t[:], pm[:], W(pref + bn_)[:])
                        transpose_block(
                            dT1x[blk * P:(blk + 1) * P, half * H:(half + 1) * H], t[:])
                for blk in range(pl.n_wb):
                    pm = pp.tile([P, P], F32, tag="ps")
                    nc.tensor.matmul(out=pm[:], lhsT=W(pref + "dg_W")[:],
                                     rhs=xT_own[:, blk * P:(blk + 1) * P],
                                     start=True, stop=True, **MM)
                    t = wp.tile([P, P], F32, tag="tb_i")
                    nc.scalar.activation(t[:], pm[:], AF.Copy)
                    pt = pp.tile([P, P], F32, tag="ps")
                    nc.tensor.transpose(out=pt[:], in_=t[:], identity=ident[:])
                    nc.scalar.activation(T2x_strip[:, blk * P:(blk + 1) * P], pt[:], AF.Copy)
            else:
                for blk in range(pl.n_eb):
                    ysl = wp.tile([P, P], F32, tag="ysl")
                    nc.sync.dma_start(out=ysl[:], in_=dyT[:, blk * P:(blk + 1) * P])
                    for half, (wn, bn_) in enumerate([("sg_", None), ("du_", "du_b")]):
                        pm = pp.tile([P, P], F32, tag="ps")
                        nc.tensor.matmul(out=pm[:], lhsT=W(pref + wn + "W")[:],
                                         rhs=ysl[:], start=True, stop=True, **MM)
                        t = wp.tile([P, P], F32, tag="tb_i")
                        if bn_ is None:
                            nc.scalar.activation(t[:], pm[:], AF.Copy)
                        else:
                            nc.vector.tensor_scalar_add(t[:], pm[:], W(pref + bn_)[:])
                        transpose_block(
                            dT1y_in[blk * P:(blk + 1) * P, half * H:(half + 1) * H], t[:])
                    pm = pp.tile([P, P], F32, tag="ps")
                    nc.tensor.matmul(out=pm[:], lhsT=W(pref + "dg_W")[:],
                                     rhs=ysl[:], start=True, stop=True, **MM)
                    t = wp.tile([P, P], F32, tag="tb_i")
                    nc.scalar.activation(t[:], pm[:], AF.Copy)
                    transpose_block(dT2y[blk * P:(blk + 1) * P, :], t[:])
                nc.gpsimd.collective_compute(
                    "AllGather", ALU.bypass,
                    ins=[dT1y_in[:]], outs=[dT1y[:]], replica_groups=[CORE_IDS])

            if kind == "node":
                idx_sb = pool.tile([P, ECAP // 16], mybir.dt.int16, tag="idx16")
                nc.sync.dma_start(out=idx_sb[:], in_=src_idx16[:])
            else:
                idx_sb = pool.tile([P, pl.n_tt], mybir.dt.int32, tag="idx32")
                nc.sync.dma_start(out=idx_sb[:], in_=ts_idx32[:])
            dstcol_sb = pool.tile([P, n_tiles], F32, tag="dstcol")
            nc.sync.dma_start(out=dstcol_sb[:], in_=dstcol_i[:])
            mcol_sb = pool.tile([P, n_tiles], F32, tag="mcol")
            nc.sync.dma_start(out=mcol_sb[:], in_=mcol_i[:])

            tile_wins = [[] for _ in range(n_tiles)]
            for k in range(n_win):
                lo, hi = scat[k]
                for j in range(max(lo, 0), min(hi + 1, n_tiles)):
                    tile_wins[j].append(k)
            win_first, win_last = {}, {}
            for j in range(n_tiles):
                for k in tile_wins[j]:
                    win_first.setdefault(k, j)
                    win_last[k] = j

            pstat = ppw.tile([P, 2], F32, tag="pstat")
            slot_tile = [None, None]
            slot_win = [None, None]

            def close_slot(s):
                k = slot_win[s]
                if k is None:
                    return
                wh, ws = slot_tile[s]
                # h = wh/(ws+eps) ; v = SU.state + b + h ; stats ; stash v
                den = wp.tile([P, 512], F32, tag="den")
                nc.vector.tensor_scalar_add(den[:], ws[:], 1e-6)
                nc.vector.reciprocal(den[:], den[:])
                hsl = wp.tile([P, 512], F32, tag="hsl")
                nc.vector.tensor_mul(hsl[:], wh[:], den[:])
                if kind == "node":
                    rhs_u = xT_own[:, k * 512:(k + 1) * 512]
                else:
                    rhs_u = wp.tile([P, 512], F32, tag="yupd")
                    nc.sync.dma_start(out=rhs_u[:], in_=dyT[:, k * 512:(k + 1) * 512])
                    rhs_u = rhs_u[:]
                pmu = pp.tile([P, 512], F32, tag="ps")
                nc.tensor.matmul(out=pmu[:], lhsT=W(pref + "su_W")[:], rhs=rhs_u,
                                 start=True, stop=True, **MM)
                v = wp.tile([P, 512], F32, tag="vup")
                nc.vector.tensor_scalar_add(v[:], pmu[:], W(pref + "su_b")[:])
                nc.vector.tensor_add(v[:], v[:], hsl[:])
                msk = wp.tile([P, 512], F32, tag="vmask")
                if kind == "node":
                    nc.sync.dma_start(out=msk[:], in_=nmask_bc[:, k * 512:(k + 1) * 512])
                else:
                    nc.sync.dma_start(out=msk[:], in_=emask_row_bc[:, k * 512:(k + 1) * 512])
                accum_stats(stat_a2, v[:], msk[:], P, k == 0)
                nc.sync.dma_start(out=dvb[:, k * 512:(k + 1) * 512], in_=v[:])
                slot_win[s] = None

            def open_slot(k):
                s = k % 2
                close_slot(s)
                slot_win[s] = k
                wh = ppw.tile([P, 512], F32, tag=f"pwh{s}")
                ws = ppw.tile([P, 512], F32, tag=f"pws{s}")
                slot_tile[s] = (wh, ws)

            _gp_cm = tc.tile_pool(name="g" + pref, bufs=2)
            gp = _gp_cm.__enter__()
            for q in range(0, n_tiles, 8):
                gt = gp.tile([P, 8 * 2 * H], F32, tag="gt")
                if kind == "node":
                    nc.gpsimd.dma_gather(
                        out_ap=gt[:].rearrange("p (j e) -> p j e", e=2 * H),
                        in_ap=dT1x[:],
                        idxs_ap=idx_sb[:, q * 8:q * 8 + 64],
                        num_idxs=1024, num_idxs_reg=1024, elem_size=2 * H)
                else:
                    for jj in range(8):
                        j = q + jj
                        nc.gpsimd.indirect_dma_start(
                            out=gt[:, jj * 2 * H:(jj + 1) * 2 * H],
                            out_offset=None, in_=dT1y[:],
                            in_offset=bass.IndirectOffsetOnAxis(
                                ap=idx_sb[:, j:j + 1], axis=0))
                for jj in range(8):
                    j = q + jj
                    if j >= n_tiles:
                        break
                    gA = gt[:, jj * 2 * H: jj * 2 * H + H]
                    gB = gt[:, jj * 2 * H + H: (jj + 1) * 2 * H]
                    pm = pp.tile([P, P], F32, tag="ps")
                    first = True
                    dr = wp.tile([P, P], F32, tag="dstrow")
                    nc.sync.dma_start(out=dr[:], in_=dstrow_i[:, j * P:(j + 1) * P])
                    for b in blocks[j]:
                        tloc = wp.tile([P, P], F32, tag="tloc")
                        nc.vector.tensor_scalar(tloc[:], dr[:], float(b * P), None,
                                                ALU.subtract)
                        s_we = wp.tile([P, P], F32, tag="s_we")
                        nc.vector.tensor_tensor(
                            out=s_we[:], in0=iotaP[:].to_broadcast([P, P]),
                            in1=tloc[:], op=ALU.is_equal)
                        if kind == "node":
                            rhs_blk = T2x_strip[:, b * P:(b + 1) * P]
                        else:
                            t2b = wp.tile([P, P], F32, tag="t2b")
                            nc.sync.dma_start(out=t2b[:], in_=dT2y[b * P:(b + 1) * P, :])
                            rhs_blk = t2b[:]
                        nc.tensor.matmul(out=pm[:], lhsT=s_we[:], rhs=rhs_blk,
                                         start=first, stop=False, **MM)
                        first = False
                    eg = wp.tile([P, P], F32, tag="egt")
                    nc.sync.dma_start(out=eg[:], in_=edge_d[:, j * P:(j + 1) * P])
                    nc.tensor.matmul(out=pm[:], lhsT=eg[:], rhs=W(pref + "eg_W")[:],
                                     start=first, stop=True, **MM)
                    m_sb = wp.tile([P, P], F32, tag="m_sb")
                    nc.vector.tensor_add(m_sb[:], pm[:], gA)
                    nc.vector.tensor_add(m_sb[:], m_sb[:], bm_bc[:])
                    sig = wp.tile([P, P], F32, tag="sig")
                    nc.scalar.activation(sig[:], m_sb[:], AF.Sigmoid)
                    bhs = wp.tile([P, P], F32, tag="bhs")
                    nc.vector.tensor_mul(bhs[:], gB, sig[:])
                    msq = wp.tile([P, P], F32, tag="msq")
                    nc.vector.tensor_mul(msq[:], m_sb[:], m_sb[:])
                    nc.tensor.matmul(out=pstat[:, 0:1], lhsT=m_sb[:],
                                     rhs=mcol_sb[:, j:j + 1],
                                     start=(j == 0), stop=(j == n_tiles - 1), **MM)
                    nc.tensor.matmul(out=pstat[:, 1:2], lhsT=msq[:],
                                     rhs=mcol_sb[:, j:j + 1],
                                     start=(j == 0), stop=(j == n_tiles - 1), **MM)
                    ptm = pp.tile([P, P], F32, tag="ps")
                    nc.tensor.transpose(out=ptm[:], in_=m_sb[:], identity=ident[:])
                    mto = wp.tile([P, P], F32, tag="mto")
                    nc.scalar.activation(mto[:], ptm[:], AF.Copy)
                    nc.sync.dma_start(out=dmT[:, j * P:(j + 1) * P], in_=mto[:])
                    for k in tile_wins[j]:
                        s = k % 2
                        if slot_win[s] != k:
                            open_slot(k)
                        wh, ws = slot_tile[s]
                        tl2 = wp.tile([P, 1], F32, tag="dcol")
                        nc.vector.tensor_scalar(tl2[:], dstcol_sb[:, j:j + 1],
                                                float(k * 512), None, ALU.subtract)
                        s_ew = wp.tile([P, 512], F32, tag="s_ew")
                        nc.vector.tensor_tensor(
                            out=s_ew[:], in0=tl2[:].to_broadcast([P, 512]),
                            in1=iota_t[:], op=ALU.is_equal)
                        nc.tensor.matmul(out=wh[:], lhsT=bhs[:], rhs=s_ew[:],
                                         start=(win_first[k] == j),
                                         stop=(win_last[k] == j), **MM)
                        nc.tensor.matmul(out=ws[:], lhsT=sig[:], rhs=s_ew[:],
                                         start=(win_first[k] == j),
                                         stop=(win_last[k] == j), **MM)
            for s in (0, 1):
                close_slot(s)
            _gp_cm.__exit__(None, None, None)

            nc.vector.tensor_copy(stat_a1[:], pstat[:])
            do_ar([(stat_a1, P, 0), (stat_a2, P, 2)])
            nc.vector.tensor_copy(stat_a1[:], ar_out_sb[:, 0:2])
            nc.vector.tensor_copy(stat_a2[:], ar_out_sb[:, 2:4])

            # node-state update: silu(bn(v)) + residual
            finish_bn(stat_a2, count_n, W(pref + "bnn_g")[:], W(pref + "bnn_b")[:])
            for k in range(WIN // 512):
                v = wp.tile([P, 512], F32, tag="vup")
                nc.sync.dma_start(out=v[:], in_=dvb[:, k * 512:(k + 1) * 512])
                nc.scalar.activation(v[:], v[:], AF.Silu, bias=bi[:], scale=sc[:])
                if kind == "node":
                    nc.vector.tensor_add(xT_own[:, k * 512:(k + 1) * 512],
                                         xT_own[:, k * 512:(k + 1) * 512], v[:])
                else:
                    yo = wp.tile([P, 512], F32, tag="yupd")
                    nc.sync.dma_start(out=yo[:], in_=dyT[:, k * 512:(k + 1) * 512])
                    nc.vector.tensor_add(yo[:], yo[:], v[:])
                    nc.sync.dma_start(out=dyT[:, k * 512:(k + 1) * 512], in_=yo[:])

            # edge-state update from dmT
            finish_bn(stat_a1, count_e, W(pref + "bne_g")[:], W(pref + "bne_b")[:])
            for c in range(CAP // 512):
                mv = wp.tile([P, 512], F32, tag="mvt")
                nc.sync.dma_start(out=mv[:], in_=dmT[:, c * 512:(c + 1) * 512])
                nc.scalar.activation(mv[:], mv[:], AF.Silu, bias=bi[:], scale=sc[:])
                es = wp.tile([P, 512], F32, tag="est")
                nc.sync.dma_start(out=es[:], in_=edge_d[:, c * 512:(c + 1) * 512])
                nc.vector.tensor_add(es[:], es[:], mv[:])
                nc.sync.dma_start(out=edge_d[:, c * 512:(c + 1) * 512], in_=es[:])

            if kind == "node":
                # allgather x
                nc.sync.dma_start(out=dxag_in[:], in_=xT_own[:, 0:WCAP])
                nc.gpsimd.collective_compute(
                    "AllGather", ALU.bypass,
                    ins=[dxag_in[:]], outs=[dxag[:]], replica_groups=[CORE_IDS])
                dx3 = dxag[:].rearrange("(d p) w -> d p w", p=P)
                for d in range(NC):
                    wd = int(pl.W_d[d])
                    if wd > 0:
                        nc.sync.dma_start(
                            out=xT[:, int(pl.node_cut[d]):int(pl.node_cut[d]) + wd],
                            in_=dx3[d, :, 0:wd])

        for pref, kind in pl.layer_prefixes:
            egcn_layer(pref, kind)

        # ============================================================ readout
        gm3 = G_mat[:].rearrange("(c p) g -> c p g", p=P)
        ph = ppw.tile([P, G], F32, tag="pwh0")
        for c in range(N // P):
            gmc = wp.tile([P, G], F32, tag="den")
            nc.sync.dma_start(out=gmc[:], in_=gm3[c])
            ptx = pp.tile([P, P], F32, tag="ps")
            nc.tensor.transpose(out=ptx[:], in_=xT[:, c * P:(c + 1) * P], identity=ident[:])
            xrow = wp.tile([P, P], F32, tag="xrow")
            nc.scalar.activation(xrow[:], ptx[:], AF.Copy)
            nc.tensor.matmul(out=ph[:], lhsT=xrow[:], rhs=gmc[:],
                             start=(c == 0), stop=(c == N // P - 1), **MM)
        hg = pool.tile([P, G], F32, tag="hg")
        nc.scalar.activation(hg[:], ph[:], AF.Copy)
        pfc = ppw.tile([G, 1], F32, tag="pws0")
        nc.tensor.matmul(out=pfc[:], lhsT=hg[:], rhs=W("fc_W")[:], start=True, stop=True, **MM)
        og = wp.tile([G, 1], F32, tag="og")
        nc.vector.tensor_add(og[:], pfc[:], W("fc_b")[:G])
        nc.sync.dma_start(out=out_p[:], in_=og[:])

        _stk.close()

    nc.finalize()
    return nc


def _in_maps(pl):
    shared = {
        "afT": pl.afT, "G_mat": pl.G_mat, "iota512_bc": pl.iota512_bc,
        "iotaP_col": pl.iotaP_col, "iotaN_col": pl.iotaN_col,
        "e_centers": pl.e_centers, "a_centers": pl.a_centers,
    }
    for k, v in pl.params.items():
        shared["p_" + k] = v
    maps = []
    for d in range(NC):
        dv = pl.dev[d]
        m = dict(shared)
        m.update({
            "src_idx16": dv.src_idx16, "dst_l_col": dv.dst_l_col,
            "emask_col": dv.emask_col, "dst_row_bc": dv.dst_row_bc,
            "emask_row_bc": dv.emask_row_bc,
            "ts_idx32": dv.ts_idx32, "tdst_l_col": dv.tdst_l_col,
            "tmask_col": dv.tmask_col, "tdst_row_bc": dv.tdst_row_bc,
            "tmask_row_bc": dv.tmask_row_bc,
            "bond_bc": dv.bond_bc, "angle_bc": dv.angle_bc,
            "nmask_bc": dv.nmask_bc, "ncut_bc": dv.ncut_bc,
        })
        maps.append(m)
    return maps


def _np_tree(v):
    if isinstance(v, dict):
        return {k: _np_tree(x) for k, x in v.items()}
    if isinstance(v, (list, tuple)):
        return [_np_tree(x) for x in v]
    return np.asarray(v)


def kernel(atom_features, bondlength, angle_features, params,
           edge_src, edge_dst, t_src, t_dst, node2graph):
    atom_features = np.asarray(atom_features, np.float32)
    bondlength = np.asarray(bondlength, np.float32)
    angle_features = np.asarray(angle_features, np.float32)
    edge_src = np.asarray(edge_src)
    edge_dst = np.asarray(edge_dst)
    t_src = np.asarray(t_src)
    t_dst = np.asarray(t_dst)
    node2graph = np.asarray(node2graph)
    params = _np_tree(params)
    pl = build_plan(atom_features, bondlength, angle_features, params,
                    edge_src, edge_dst, t_src, t_dst, node2graph)
    ncb = build_program(pl)
    res = run_bass_kernel_spmd(
        ncb, _in_maps(pl), list(range(NC)),
        trace=os.environ.get("BASS_KERNEL_TRACE", "0") == "1")
    kernel.last_exec_time_ns = res.exec_time_ns
    out = np.asarray(res.results[0]["out"]).reshape(-1)[:pl.G]
    return out.astype(np.float32)


kernel.last_exec_time_ns = None


# revision 10
# speedup vs baseline: 1.2777x; 1.2777x over previous
"""ALIGNN forward pass on 8 trn2 NeuronCores (Bass/Tile).

Sharding: edges sorted by dst with node-aligned cuts; triplets sorted by
(renumbered) t_dst with edge-aligned cuts. Device d owns a node range, an
edge range and a triplet range. x replicated (block AllGather per layer);
y sharded (gather table allgathered row-major per line layer); z local.

Message pass per egcn: src side via one random row gather of a pre-fused
[A.s || B.s + b] row-major table (dma_gather for the 8192-row node table,
indirect_dma_start for the allgathered 8*E_CAP-row edge table); dst side and
segment_sum via one-hot window matmuls on PE (dst-sorted => dense windows);
BatchNorm via masked matmul stats + one batched AllReduce per layer.
"""
import contextlib
import ctypes
import os
import sys
import types

import numpy as np

# ----------------------------------------------------------- env shims
# (kernel.py must be self-contained: these patch the local runtime only)


def _install_profile_shim():
    try:
        if 'antenv.axon_hooks' not in sys.modules:
            mod = types.ModuleType('antenv.axon_hooks')
            _hook = [None]
            mod.set_axon_ntff_profile_hook = lambda h: _hook.__setitem__(0, h)
            mod.get_axon_ntff_profile_hook = lambda: _hook[0]
            sys.modules['antenv.axon_hooks'] = mod
            import antenv
            antenv.axon_hooks = mod
        from antenv.axon_hooks import set_axon_ntff_profile_hook
        lib = ctypes.CDLL('/opt/axon/libaxon_pjrt.so')
        if not hasattr(lib, 'axon_start_nrt_profile'):
            return
        lib.axon_start_nrt_profile.argtypes = [
            ctypes.POINTER(ctypes.c_int64), ctypes.c_size_t]
        lib.axon_start_nrt_profile.restype = ctypes.c_int64
        lib.axon_stop_nrt_profile.argtypes = [ctypes.c_char_p]
        lib.axon_stop_nrt_profile.restype = ctypes.c_int64
        lib.axon_reset.restype = ctypes.c_int64

        @contextlib.contextmanager
        def robust_hook(output_dir, device_ids):
            import jax
            jax.devices()

            def _start():
                if device_ids:
                    ids = (ctypes.c_int64 * len(device_ids))(*device_ids)
                    return lib.axon_start_nrt_profile(ids, len(device_ids))
                return lib.axon_start_nrt_profile(None, 0)

            rc = _start()
            if rc != 0:
                lib.axon_reset()
                rc = _start()
            if rc != 0:
                raise RuntimeError(f"axon_start_nrt_profile rc={rc}")
            try:
                yield
            finally:
                n = lib.axon_stop_nrt_profile(str(output_dir).encode())
                if n < 0:
                    lib.axon_reset()
                    raise RuntimeError(f"axon_stop_nrt_profile rc={n}")

        set_axon_ntff_profile_hook(robust_hook)
        from concourse import bass_utils
        bass_utils.upload_artifacts = lambda tmpdir: f"local:{tmpdir}"
    except Exception:
        pass


def _install_drain_patch():
    # this environment's walrus rejects >1 sync wait per instruction; split
    # the kernel-tail drain's waits across one nop per logical proc.
    from concourse import tile as _tile
    from concourse.vector_clock import ScopedClock

    def _patched(self, tick_clock, wait_clock):
        vc = tick_clock.global_clock
        for proc in range(len(vc)):
            t = vc[proc]
            if t > 0:
                nop = self.nc.sync.nop(nofuse=True, hint=f"drain_split_{proc}")
                pc = ScopedClock()
                pc.require_at_least(None, proc, t)
                wait_clock.add_sem_waits(nop.ins, pc)
        drain_inst = self.nc.sync.drain()
        wait_clock.add_sem_waits(
            drain_inst.ins, ScopedClock({None: tick_clock.global_clock}))
        si = drain_inst.ins.sync_info
        if si is not None and len(si.on_wait) > 1:
            si.on_wait = [si.on_wait[0]]
        self.nc.all_engine_barrier()
        assert self.sems is not None
        popped = self.nc._tile_sem_poison_stack.pop()
        assert popped is self._sem_poison
        self.nc.clear_and_free_semaphores(list(self.sems.allocated().values()))
        self.nc.all_engine_barrier()

    _tile.TileContext._drain_and_barrier = _patched


_install_profile_shim()
_install_drain_patch()

from concourse import bass, bacc, mybir, tile
from concourse.bass_utils import run_bass_kernel_spmd
from concourse.masks import make_identity

P = 128
H = 128
NC = 8
F32 = mybir.dt.float32
EPS_BN = 1e-5
AF = mybir.ActivationFunctionType
ALU = mybir.AluOpType


def _ceil_to(x, m):
    return ((x + m - 1) // m) * m


class _O:
    pass


# ------------------------------------------------------------------ host plan

def build_plan(atom_features, bondlength, angle_features, params,
               edge_src, edge_dst, t_src, t_dst, node2graph):
    pl = _O()
    N = atom_features.shape[0]
    E = edge_src.shape[0]
    T = t_src.shape[0]
    G = int(node2graph.max()) + 1
    pl.N, pl.E, pl.T, pl.G = N, E, T, G
    EB, AB = 80, 40
    pl.EB, pl.AB = EB, AB

    e_perm = np.argsort(edge_dst, kind="stable")
    sdst = edge_dst[e_perm]
    ssrc = edge_src[e_perm]
    cut_e = np.zeros(NC + 1, dtype=np.int64)
    for d in range(1, NC):
        c = d * E // NC
        while 0 < c < E and sdst[c] == sdst[c - 1]:
            c += 1
        cut_e[d] = c
    cut_e[NC] = E
    node_cut = np.zeros(NC + 1, dtype=np.int64)
    for d in range(1, NC):
        node_cut[d] = sdst[cut_e[d]] if cut_e[d] < E else N
    node_cut[NC] = N
    E_d = np.diff(cut_e)
    W_d = np.diff(node_cut)
    pl.cut_e, pl.node_cut, pl.E_d, pl.W_d = cut_e, node_cut, E_d, W_d
    pl.E_CAP = _ceil_to(int(E_d.max()), 1024)
    pl.W_CAP = _ceil_to(int(W_d.max()) + 1, 512)

    inv_e = np.empty(E, dtype=np.int64)
    inv_e[e_perm] = np.arange(E)
    ts_new = inv_e[t_src]
    td_new = inv_e[t_dst]
    t_perm = np.argsort(td_new, kind="stable")
    sts = ts_new[t_perm]
    std = td_new[t_perm]
    cut_t = np.searchsorted(std, cut_e).astype(np.int64)
    cut_t[0], cut_t[NC] = 0, T
    T_d = np.diff(cut_t)
    pl.cut_t, pl.T_d = cut_t, T_d
    pl.T_CAP = _ceil_to(int(T_d.max()), 1024)

    ECAP, TCAP, WCAP = pl.E_CAP, pl.T_CAP, pl.W_CAP
    n_et, n_tt = ECAP // P, TCAP // P
    pl.n_et, pl.n_tt = n_et, n_tt
    pl.n_wb, pl.n_eb = WCAP // P, ECAP // P
    pl.n_wwin, pl.n_ewin = WCAP // 512, ECAP // 512

    e_blocks = [set() for _ in range(n_et)]
    e_scat = [[n_et, -1] for _ in range(pl.n_wwin)]
    t_blocks = [set() for _ in range(n_tt)]
    t_scat = [[n_tt, -1] for _ in range(pl.n_ewin)]
    pl.dev = []
    for d in range(NC):
        dv = _O()
        el, eh = cut_e[d], cut_e[d + 1]
        ne = int(eh - el)
        src_g = np.full(ECAP, ssrc[eh - 1] if ne else 0, dtype=np.int64)
        src_g[:ne] = ssrc[el:eh]
        dst_l = np.full(ECAP, 2.0 * WCAP, dtype=np.float32)
        dst_l[:ne] = (sdst[el:eh] - node_cut[d]).astype(np.float32)
        emask = np.zeros(ECAP, dtype=np.float32)
        emask[:ne] = 1.0
        dv.src_idx16 = np.tile(
            src_g.astype(np.int16).reshape(ECAP // 16, 16).T, (8, 1)).copy()
        dv.dst_l_col = dst_l.reshape(n_et, P).T.copy()
        dv.emask_col = emask.reshape(n_et, P).T.copy()
        dv.dst_row_bc = np.tile(dst_l[None, :], (P, 1))
        dv.emask_row_bc = np.tile(emask[None, :], (P, 1))
        for j in range(n_et):
            dd = dst_l[j * P:(j + 1) * P]
            rl = dd[dd < WCAP]
            if rl.size:
                lo, hi = int(rl.min()), int(rl.max())
                for b in range(lo // P, hi // P + 1):
                    e_blocks[j].add(b)
                for k in range(lo // 512, hi // 512 + 1):
                    e_scat[k][0] = min(e_scat[k][0], j)
                    e_scat[k][1] = max(e_scat[k][1], j)

        tl, th = cut_t[d], cut_t[d + 1]
        nt = int(th - tl)
        ts_l = np.full(TCAP, sts[th - 1] if nt else 0, dtype=np.int64)
        ts_l[:nt] = sts[tl:th]
        tdl = np.full(TCAP, 2.0 * ECAP, dtype=np.float32)
        tdl[:nt] = (std[tl:th] - cut_e[d]).astype(np.float32)
        tmask = np.zeros(TCAP, dtype=np.float32)
        tmask[:nt] = 1.0
        owner = np.clip(np.searchsorted(cut_e, ts_l, side="right") - 1, 0, NC - 1)
        gidx = (owner * ECAP + (ts_l - cut_e[owner])).astype(np.int32)
        dv.ts_idx32 = gidx.reshape(n_tt, P).T.copy()
        dv.tdst_l_col = tdl.reshape(n_tt, P).T.copy()
        dv.tmask_col = tmask.reshape(n_tt, P).T.copy()
        dv.tdst_row_bc = np.tile(tdl[None, :], (P, 1))
        dv.tmask_row_bc = np.tile(tmask[None, :], (P, 1))
        for j in range(n_tt):
            dd = tdl[j * P:(j + 1) * P]
            rl = dd[dd < ECAP]
            if rl.size:
                lo, hi = int(rl.min()), int(rl.max())
                for b in range(lo // P, hi // P + 1):
                    t_blocks[j].add(b)
                for k in range(lo // 512, hi // 512 + 1):
                    t_scat[k][0] = min(t_scat[k][0], j)
                    t_scat[k][1] = max(t_scat[k][1], j)

        bl = np.zeros(ECAP, dtype=np.float32)
        bl[:ne] = bondlength[e_perm[el:eh]]
        dv.bond_bc = np.tile(bl[None, :], (EB, 1))
        af = np.zeros(TCAP, dtype=np.float32)
        af[:nt] = angle_features[t_perm[tl:th]]
        dv.angle_bc = np.tile(af[None, :], (AB, 1))
        nm = np.zeros(WCAP, dtype=np.float32)
        nm[:W_d[d]] = 1.0
        dv.nmask_bc = np.tile(nm[None, :], (P, 1))
        dv.ncut_bc = np.tile(
            (np.arange(WCAP, dtype=np.float32) + node_cut[d])[None, :], (P, 1))
        pl.dev.append(dv)

    pl.e_blocks = [sorted(b) for b in e_blocks]
    pl.e_scat = e_scat
    pl.t_blocks = [sorted(b) for b in t_blocks]
    pl.t_scat = t_scat

    def chunk_ranges(blocks, n_tiles):
        rngs = []
        for q in range(0, n_tiles, 8):
            bs = [b for j in range(q, min(q + 8, n_tiles)) for b in blocks[j]]
            if not bs:
                bs = [0]
            rngs.append((min(bs), max(bs)))
        return rngs
    pl.t_chunk_rng = chunk_ranges(pl.t_blocks, n_tt)
    pl.t_max_nbl = max(hi - lo + 1 for lo, hi in pl.t_chunk_rng)

    pl.afT = np.ascontiguousarray(atom_features.T.astype(np.float32))
    counts = np.bincount(node2graph, minlength=G).astype(np.float32)
    Gm = np.zeros((N, G), dtype=np.float32)
    Gm[np.arange(N), node2graph] = 1.0 / np.maximum(counts[node2graph], 1.0)
    pl.G_mat = Gm
    pl.iota512_bc = np.tile(np.arange(512, dtype=np.float32)[None, :], (P, 1))
    pl.iotaP_col = np.arange(P, dtype=np.float32).reshape(P, 1)
    pl.iotaN_col = np.arange(N, dtype=np.float32).reshape(N // P, P).T.copy()

    pr = {}

    def addlin(pref, p):
        pr[pref + "W"] = np.asarray(p["W"], np.float32)
        pr[pref + "b"] = np.asarray(p["b"], np.float32).reshape(-1, 1)

    def addmlp(pref, p):
        addlin(pref, p)
        pr[pref + "g"] = np.asarray(p["gamma"], np.float32).reshape(-1, 1)
        pr[pref + "be"] = np.asarray(p["beta"], np.float32).reshape(-1, 1)

    addmlp("atom_", params["atom_emb"])
    addmlp("eemb0_", params["edge_emb"][0])
    addmlp("eemb1_", params["edge_emb"][1])
    addmlp("aemb0_", params["angle_emb"][0])
    addmlp("aemb1_", params["angle_emb"][1])

    def addegcn(pref, p):
        addlin(pref + "sg_", p["src_gate"])
        addlin(pref + "dg_", p["dst_gate"])
        addlin(pref + "eg_", p["edge_gate"])
        addlin(pref + "su_", p["src_update"])
        addlin(pref + "du_", p["dst_update"])
        for nm_, k in [("bnn_g", "bn_nodes_g"), ("bnn_b", "bn_nodes_b"),
                       ("bne_g", "bn_edges_g"), ("bne_b", "bn_edges_b")]:
            pr[pref + nm_] = np.asarray(p[k], np.float32).reshape(-1, 1)

    pl.layer_prefixes = []
    lix = 0
    for lp in params["alignn"]:
        addegcn(f"l{lix}_", lp["node"]); pl.layer_prefixes.append((f"l{lix}_", "node")); lix += 1
        addegcn(f"l{lix}_", lp["edge"]); pl.layer_prefixes.append((f"l{lix}_", "line")); lix += 1
    for p in params["gcn"]:
        addegcn(f"l{lix}_", p); pl.layer_prefixes.append((f"l{lix}_", "node")); lix += 1
    addlin("fc_", params["fc"])
    pr["fc_b"] = np.tile(np.asarray(params["fc"]["b"], np.float32).reshape(1, 1), (P, 1))
    for pref, _ in pl.layer_prefixes:
        bm = (pr[pref + "sg_b"] + pr[pref + "dg_b"] + pr[pref + "eg_b"]).reshape(-1)
        pr[pref + "bm_bc"] = np.tile(bm[None, :], (P, 1)).astype(np.float32)
    pl.params = pr

    pl.e_centers = np.linspace(0.0, 8.0, EB, dtype=np.float32).reshape(-1, 1)
    pl.e_gamma = float(1.0 / (8.0 / (EB - 1)))
    pl.a_centers = np.linspace(-1.0, 1.0, AB, dtype=np.float32).reshape(-1, 1)
    pl.a_gamma = float(1.0 / (2.0 / (AB - 1)))
    return pl


# ------------------------------------------------------------------ program

def build_program(pl):
    nc = bacc.Bacc("TRN2")
    N, E, T, G = pl.N, pl.E, pl.T, pl.G
    EB, AB = pl.EB, pl.AB
    ECAP, TCAP, WCAP = pl.E_CAP, pl.T_CAP, pl.W_CAP
    CORE_IDS = list(range(NC))

    def inp(name, shape, dt=F32):
        return nc.declare_dram_parameter(name, list(shape), dt, isOutput=False)

    src_idx16 = inp("src_idx16", [P, ECAP // 16], mybir.dt.int16)
    dst_l_col = inp("dst_l_col", [P, pl.n_et])
    emask_col = inp("emask_col", [P, pl.n_et])
    dst_row_bc = inp("dst_row_bc", [P, ECAP])
    emask_row_bc = inp("emask_row_bc", [P, ECAP])
    ts_idx32 = inp("ts_idx32", [P, pl.n_tt], mybir.dt.int32)
    tdst_l_col = inp("tdst_l_col", [P, pl.n_tt])
    tmask_col = inp("tmask_col", [P, pl.n_tt])
    tdst_row_bc = inp("tdst_row_bc", [P, TCAP])
    tmask_row_bc = inp("tmask_row_bc", [P, TCAP])
    bond_bc = inp("bond_bc", [EB, ECAP])
    angle_bc = inp("angle_bc", [AB, TCAP])
    nmask_bc = inp("nmask_bc", [P, WCAP])
    ncut_bc = inp("ncut_bc", [P, WCAP])
    afT_i = inp("afT", [pl.afT.shape[0], N])
    G_mat = inp("G_mat", [N, G])
    iota512_i = inp("iota512_bc", [P, 512])
    iotaP_i = inp("iotaP_col", [P, 1])
    iotaN_i = inp("iotaN_col", [P, N // P])
    e_cent = inp("e_centers", [EB, 1])
    a_cent = inp("a_centers", [AB, 1])
    prm = {k: inp("p_" + k, v.shape) for k, v in pl.params.items()}
    out_p = nc.declare_dram_parameter("out", [G, 1], F32, isOutput=True)

    dT1x = nc.dram_tensor("dT1x", [N, 2 * H], F32)
    dT2y = nc.dram_tensor("dT2y", [ECAP, H], F32)
    dT1y_in = nc.dram_tensor("dT1y_in", [ECAP, 2 * H], F32)
    dT1y = nc.dram_tensor("dT1y", [NC * ECAP, 2 * H], F32, addr_space="Shared")
    dyT = nc.dram_tensor("dyT", [P, ECAP], F32)
    dzT = nc.dram_tensor("dzT", [P, TCAP], F32)
    dmT = nc.dram_tensor("dmT", [P, TCAP], F32)
    dvb = nc.dram_tensor("dvb", [P, max(WCAP, ECAP)], F32)
    dv0e = nc.dram_tensor("dv0e", [64, ECAP], F32)
    dv0a = nc.dram_tensor("dv0a", [64, TCAP], F32)
    dxag_in = nc.dram_tensor("dxag_in", [P, WCAP], F32)
    dxag = nc.dram_tensor("dxag", [NC * P, WCAP], F32, addr_space="Shared")
    dar_in = nc.dram_tensor("dar_in", [P, 4], F32)
    dar = nc.dram_tensor("dar", [P, 4], F32, addr_space="Shared")

    MM = dict(skip_group_check=True)

    from contextlib import ExitStack
    _stk = ExitStack()
    with tile.TileContext(nc) as tc:
        pool = _stk.enter_context(tc.tile_pool(name="main", bufs=1))
        wp = _stk.enter_context(tc.tile_pool(name="work", bufs=3))
        pp = _stk.enter_context(tc.tile_pool(name="psA", bufs=2, space="PSUM"))
        ppw = _stk.enter_context(tc.tile_pool(name="psW", bufs=1, space="PSUM"))

        ident = pool.tile([P, P], F32, tag="ident")
        make_identity(nc, ident[:])
        xT = pool.tile([P, N], F32, tag="xT")
        xT_own = pool.tile([P, WCAP], F32, tag="xT_own")
        T2x_strip = pool.tile([P, pl.n_wb * P], F32, tag="T2x")
        iota_t = pool.tile([P, 512], F32, tag="iota")
        nc.sync.dma_start(out=iota_t[:], in_=iota512_i[:])
        iotaP = pool.tile([P, 1], F32, tag="iotaP")
        nc.sync.dma_start(out=iotaP[:], in_=iotaP_i[:])
        iotaN = pool.tile([P, N // P], F32, tag="iotaN")
        nc.sync.dma_start(out=iotaN[:], in_=iotaN_i[:])

        params_sb = {}

        def W(name):
            if name not in params_sb:
                t = pool.tile(list(pl.params[name].shape), F32, tag="p_" + name)
                nc.sync.dma_start(out=t[:], in_=prm[name][:])
                params_sb[name] = t
            return params_sb[name]

        mu = pool.tile([P, 1], F32, tag="mu")
        rs = pool.tile([P, 1], F32, tag="rs")
        sc = pool.tile([P, 1], F32, tag="sc")
        bi = pool.tile([P, 1], F32, tag="bi")
        tmp1 = pool.tile([P, 1], F32, tag="tmp1")

        def finish_bn(stat_ap, count, gamma, beta, Q=P):
            nc.vector.tensor_scalar_mul(mu[:Q], stat_ap[:Q, 0:1], 1.0 / count)
            nc.vector.tensor_scalar_mul(rs[:Q], stat_ap[:Q, 1:2], 1.0 / count)
            nc.vector.tensor_mul(tmp1[:Q], mu[:Q], mu[:Q])
            nc.vector.tensor_sub(rs[:Q], rs[:Q], tmp1[:Q])
            nc.vector.tensor_scalar_add(rs[:Q], rs[:Q], EPS_BN)
            nc.scalar.activation(rs[:Q], rs[:Q], AF.Sqrt)
            nc.vector.reciprocal(rs[:Q], rs[:Q])
            nc.vector.tensor_mul(sc[:Q], gamma, rs[:Q])
            nc.vector.tensor_mul(bi[:Q], mu[:Q], sc[:Q])
            nc.vector.tensor_sub(bi[:Q], beta, bi[:Q])

        ar_in_sb = pool.tile([P, 4], F32, tag="ar_in")
        ar_out_sb = pool.tile([P, 4], F32, tag="ar_out")

        def do_ar(stats_list):
            nc.vector.memset(ar_in_sb[:], 0.0)
            for ap_, Q, co in stats_list:
                nc.vector.tensor_copy(ar_in_sb[:Q, co:co + 2], ap_[:Q, :])
            nc.sync.dma_start(out=dar_in[:], in_=ar_in_sb[:])
            nc.gpsimd.collective_compute(
                "AllReduce", ALU.add,
                ins=[dar_in[:]], outs=[dar[:]], replica_groups=[CORE_IDS])
            nc.sync.dma_start(out=ar_out_sb[:], in_=dar[:])

        stat_a1 = pool.tile([P, 2], F32, tag="stat_a1")
        stat_a2 = pool.tile([P, 2], F32, tag="stat_a2")

        def accum_stats2(stat, v_ap, mask_ap, first):
            t = wp.tile([P, 8 * P], F32, tag="ms_w")
            r = wp.tile([P, 2], F32, tag="ms_r2")
            nc.vector.tensor_mul(t[:], v_ap, mask_ap)
            nc.vector.reduce_sum(r[:, 0:1], t[:], axis=mybir.AxisListType.X)
            nc.vector.tensor_mul(t[:], t[:], v_ap)
            nc.vector.reduce_sum(r[:, 1:2], t[:], axis=mybir.AxisListType.X)
            if first:
                nc.vector.tensor_copy(stat[:, :], r[:, :])
            else:
                nc.vector.tensor_add(stat[:, :], stat[:, :], r[:, :])

        def accum_stats(stat, v_ap, mask_ap, Q, first):
            # stat[:,0]+=sum(v*mask) ; stat[:,1]+=sum(v^2*mask)  (free-dim reduce)
            t = wp.tile([P, 512], F32, tag="ms_t")
            r = wp.tile([P, 2], F32, tag="ms_r")
            nc.vector.tensor_mul(t[:Q], v_ap, mask_ap)
            nc.vector.reduce_sum(r[:Q, 0:1], t[:Q], axis=mybir.AxisListType.X)
            nc.vector.tensor_mul(t[:Q], t[:Q], v_ap)
            nc.vector.reduce_sum(r[:Q, 1:2], t[:Q], axis=mybir.AxisListType.X)
            if first:
                nc.vector.tensor_copy(stat[:Q, :], r[:Q, :])
            else:
                nc.vector.tensor_add(stat[:Q, :], stat[:Q, :], r[:Q, :])

        # ====================================================== atom embedding
        AFP = pl.afT.shape[0]
        for c in range(N // 512):
            afc = wp.tile([AFP, 512], F32, tag="den")
            nc.sync.dma_start(out=afc[:], in_=afT_i[:, c * 512:(c + 1) * 512])
            pm = pp.tile([P, 512], F32, tag="pm")
            nc.tensor.matmul(out=pm[:], lhsT=W("atom_W")[:],
                             rhs=afc[:], start=True, stop=True, **MM)
            nc.vector.tensor_scalar_add(xT[:, c * 512:(c + 1) * 512], pm[:],
                                        W("atom_b")[:])
        ones_mask = pool.tile([P, 512], F32, tag="ones")
        nc.vector.memset(ones_mask[:], 1.0)
        for c in range(N // 512):
            accum_stats(stat_a1, xT[:, c * 512:(c + 1) * 512], ones_mask[:], P, c == 0)
        finish_bn(stat_a1, N, W("atom_g")[:], W("atom_be")[:])
        for c in range(N // 512):
            nc.scalar.activation(xT[:, c * 512:(c + 1) * 512],
                                 xT[:, c * 512:(c + 1) * 512], AF.Silu,
                                 bias=bi[:], scale=sc[:])

        # =============================================== edge/angle embeddings
        def emb_stage0(src_i, cent_i, gmm, Q_in, CAP, w0, b0, dst_d, mask_i, stat):
            cent_sb = wp.tile([Q_in, 1], F32, tag="dcol")
            nc.sync.dma_start(out=cent_sb[:], in_=cent_i[:])
            for c in range(CAP // 512):
                sl_i = wp.tile([Q_in, 512], F32, tag="den")
                nc.sync.dma_start(out=sl_i[:], in_=src_i[:, c * 512:(c + 1) * 512])
                t = wp.tile([Q_in, 512], F32, tag="hsl")
                nc.vector.tensor_scalar(t[:], sl_i[:], cent_sb[:], None, ALU.subtract)
                nc.vector.tensor_mul(t[:], t[:], t[:])
                nc.scalar.activation(t[:], t[:], AF.Exp, scale=-gmm)
                pm = pp.tile([64, 512], F32, tag="pm")
                nc.tensor.matmul(out=pm[:], lhsT=w0[:], rhs=t[:], start=True, stop=True, **MM)
                v = wp.tile([64, 512], F32, tag="vup")
                nc.vector.tensor_scalar_add(v[:], pm[:], b0[:])
                msk = wp.tile([P, 512], F32, tag="yupd")
                nc.sync.dma_start(out=msk[:64], in_=mask_i[:64, c * 512:(c + 1) * 512])
                accum_stats(stat, v[:], msk[:64], 64, c == 0)
                nc.sync.dma_start(out=dst_d[:, c * 512:(c + 1) * 512], in_=v[:])

        emb_stage0(bond_bc, e_cent, pl.e_gamma, EB, ECAP, W("eemb0_W"),
                   W("eemb0_b"), dv0e, emask_row_bc, stat_a1)
        emb_stage0(angle_bc, a_cent, pl.a_gamma, AB, TCAP, W("aemb0_W"),
                   W("aemb0_b"), dv0a, tmask_row_bc, stat_a2)
        do_ar([(stat_a1, 64, 0), (stat_a2, 64, 2)])
        nc.vector.tensor_copy(stat_a1[:64], ar_out_sb[:64, 0:2])
        nc.vector.tensor_copy(stat_a2[:64], ar_out_sb[:64, 2:4])

        def emb_stage1(src_d, CAP, stat, count, g0, be0, w1, b1, dst_d, mask_i, stat2):
            finish_bn(stat, count, g0, be0, Q=64)
            for c in range(CAP // 512):
                v = wp.tile([64, 512], F32, tag="vup")
                nc.sync.dma_start(out=v[:], in_=src_d[:, c * 512:(c + 1) * 512])
                nc.scalar.activation(v[:], v[:], AF.Silu, bias=bi[:64], scale=sc[:64])
                pm = pp.tile([P, 512], F32, tag="pm")
                nc.tensor.matmul(out=pm[:], lhsT=w1[:], rhs=v[:], start=True, stop=True, **MM)
                v2 = wp.tile([P, 512], F32, tag="vmask")
                nc.vector.tensor_scalar_add(v2[:], pm[:], b1[:])
                msk = wp.tile([P, 512], F32, tag="yupd")
                nc.sync.dma_start(out=msk[:], in_=mask_i[:, c * 512:(c + 1) * 512])
                accum_stats(stat2, v2[:], msk[:], P, c == 0)
                nc.sync.dma_start(out=dst_d[:, c * 512:(c + 1) * 512], in_=v2[:])

        emb_stage1(dv0e, ECAP, stat_a1, E, W("eemb0_g")[:64], W("eemb0_be")[:64],
                   W("eemb1_W"), W("eemb1_b"), dyT, emask_row_bc, stat_a1)
        emb_stage1(dv0a, TCAP, stat_a2, T, W("aemb0_g")[:64], W("aemb0_be")[:64],
                   W("aemb1_W"), W("aemb1_b"), dzT, tmask_row_bc, stat_a2)
        do_ar([(stat_a1, P, 0), (stat_a2, P, 2)])
        nc.vector.tensor_copy(stat_a1[:], ar_out_sb[:, 0:2])
        nc.vector.tensor_copy(stat_a2[:], ar_out_sb[:, 2:4])

        def emb_finish(dst_d, CAP, stat, count, g1, be1):
            finish_bn(stat, count, g1, be1)
            for c in range(CAP // 512):
                v = wp.tile([P, 512], F32, tag="vmask")
                nc.sync.dma_start(out=v[:], in_=dst_d[:, c * 512:(c + 1) * 512])
                nc.scalar.activation(v[:], v[:], AF.Silu, bias=bi[:], scale=sc[:])
                nc.sync.dma_start(out=dst_d[:, c * 512:(c + 1) * 512], in_=v[:])

        emb_finish(dyT, ECAP, stat_a1, E, W("eemb1_g"), W("eemb1_be"))
        emb_finish(dzT, TCAP, stat_a2, T, W("aemb1_g"), W("aemb1_be"))

        # ------------------------------------------------ xT_own via one-hot
        def extract_own():
            for k in range(WCAP // 512):
                pxo = ppw.tile([P, 512], F32, tag="pwh0")
                ncut_c = wp.tile([P, 512], F32, tag="hsl")
                nc.sync.dma_start(out=ncut_c[:], in_=ncut_bc[:, k * 512:(k + 1) * 512])
                for cblk in range(N // P):
                    selc = wp.tile([P, 512], F32, tag="s_ew")
                    nc.vector.tensor_tensor(
                        out=selc[:], in0=iotaN[:, cblk:cblk + 1].to_broadcast([P, 512]),
                        in1=ncut_c[:], op=ALU.is_equal)
                    ptx = pp.tile([P, P], F32, tag="pm")
                    nc.tensor.transpose(out=ptx[:], in_=xT[:, cblk * P:(cblk + 1) * P],
                                        identity=ident[:])
                    xrow = wp.tile([P, P], F32, tag="xrow")
                    nc.scalar.activation(xrow[:], ptx[:], AF.Copy)
                    nc.tensor.matmul(out=pxo[:], lhsT=xrow[:], rhs=selc[:],
                                     start=(cblk == 0), stop=(cblk == N // P - 1), **MM)
                nc.scalar.activation(xT_own[:, k * 512:(k + 1) * 512], pxo[:], AF.Copy)

        extract_own()

        # ============================================================ layers
        def transpose_block(dst_dram_rows, src_fm_ap, bias=None):
            """[f,128] feature-major SBUF slice -> row-major [128,f] DRAM rows."""
            pt = pp.tile([P, P], F32, tag="pm")
            nc.tensor.transpose(out=pt[:], in_=src_fm_ap, identity=ident[:])
            t2 = wp.tile([P, P], F32, tag="tb_o")
            nc.scalar.activation(t2[:], pt[:], AF.Copy)
            nc.sync.dma_start(out=dst_dram_rows, in_=t2[:])

        def egcn_layer(pref, kind):
            if kind == "node":
                n_tiles, CAP, WIN = pl.n_et, ECAP, WCAP
                blocks, scat, n_win = pl.e_blocks, pl.e_scat, pl.n_wwin
                dstrow_i, dstcol_i, mcol_i = dst_row_bc, dst_l_col, emask_col
                count_e, count_n = E, N
                edge_d = dyT
                mask_row_i = emask_row_bc
            else:
                n_tiles, CAP, WIN = pl.n_tt, TCAP, ECAP
                blocks, scat, n_win = pl.t_blocks, pl.t_scat, pl.n_ewin
                dstrow_i, dstcol_i, mcol_i = tdst_row_bc, tdst_l_col, tmask_col
                count_e, count_n = T, E
                edge_d = dzT
                mask_row_i = tmask_row_bc
            bm_bc = W(pref + "bm_bc")

            # ---- tables
            if kind == "node":
                for blk in range(N // P):
                    for half, (wn, bn_) in enumerate([("sg_", None), ("du_", "du_b")]):
                        pm = pp.tile([P, P], F32, tag="pm")
                        nc.tensor.matmul(out=pm[:], lhsT=W(pref + wn + "W")[:],
                                         rhs=xT[:, blk * P:(blk + 1) * P],
                                         start=True, stop=True, **MM)
                        t = wp.tile([P, P], F32, tag="tb_i")
                        if bn_ is None:
                            nc.scalar.activation(t[:], pm[:], AF.Copy)
                        else:
                            nc.vector.tensor_scalar_add(t[:], pm[:], W(pref + bn_)[:])
                        transpose_block(
                            dT1x[blk * P:(blk + 1) * P, half * H:(half + 1) * H], t[:])
                for blk in range(pl.n_wb):
                    pm = pp.tile([P, P], F32, tag="pm")
                    nc.tensor.matmul(out=pm[:], lhsT=W(pref + "dg_W")[:],
                                     rhs=xT_own[:, blk * P:(blk + 1) * P],
                                     start=True, stop=True, **MM)
                    t = wp.tile([P, P], F32, tag="tb_i")
                    nc.scalar.activation(t[:], pm[:], AF.Copy)
                    pt = pp.tile([P, P], F32, tag="pm")
                    nc.tensor.transpose(out=pt[:], in_=t[:], identity=ident[:])
                    nc.scalar.activation(T2x_strip[:, blk * P:(blk + 1) * P], pt[:], AF.Copy)
            else:
                for blk in range(pl.n_eb):
                    ysl = wp.tile([P, P], F32, tag="ysl")
                    nc.sync.dma_start(out=ysl[:], in_=dyT[:, blk * P:(blk + 1) * P])
                    for half, (wn, bn_) in enumerate([("sg_", None), ("du_", "du_b")]):
                        pm = pp.tile([P, P], F32, tag="pm")
                        nc.tensor.matmul(out=pm[:], lhsT=W(pref + wn + "W")[:],
                                         rhs=ysl[:], start=True, stop=True, **MM)
                        t = wp.tile([P, P], F32, tag="tb_i")
                        if bn_ is None:
                            nc.scalar.activation(t[:], pm[:], AF.Copy)
                        else:
                            nc.vector.tensor_scalar_add(t[:], pm[:], W(pref + bn_)[:])
                        transpose_block(
                            dT1y_in[blk * P:(blk + 1) * P, half * H:(half + 1) * H], t[:])
                    pm = pp.tile([P, P], F32, tag="pm")
                    nc.tensor.matmul(out=pm[:], lhsT=W(pref + "dg_W")[:],
                                     rhs=ysl[:], start=True, stop=True, **MM)
                    t = wp.tile([P, P], F32, tag="tb_i")
                    nc.scalar.activation(t[:], pm[:], AF.Copy)
                    transpose_block(dT2y[blk * P:(blk + 1) * P, :], t[:])
                nc.gpsimd.collective_compute(
                    "AllGather", ALU.bypass,
                    ins=[dT1y_in[:]], outs=[dT1y[:]], replica_groups=[CORE_IDS])

            if kind == "node":
                idx_sb = pool.tile([P, ECAP // 16], mybir.dt.int16, tag="idx16")
                nc.sync.dma_start(out=idx_sb[:], in_=src_idx16[:])
            else:
                idx_sb = pool.tile([P, pl.n_tt], mybir.dt.int32, tag="idx32")
                nc.sync.dma_start(out=idx_sb[:], in_=ts_idx32[:])
            dstcol_sb = pool.tile([P, n_tiles], F32, tag="dstcol")
            nc.sync.dma_start(out=dstcol_sb[:], in_=dstcol_i[:])
            mcol_sb = pool.tile([P, n_tiles], F32, tag="mcol")
            nc.sync.dma_start(out=mcol_sb[:], in_=mcol_i[:])

            tile_wins = [[] for _ in range(n_tiles)]
            for k in range(n_win):
                lo, hi = scat[k]
                for j in range(max(lo, 0), min(hi + 1, n_tiles)):
                    tile_wins[j].append(k)
            win_first, win_last = {}, {}
            for j in range(n_tiles):
                for k in tile_wins[j]:
                    win_first.setdefault(k, j)
                    win_last[k] = j

            slot_tile = [None, None]
            slot_win = [None, None]

            def close_slot(s):
                k = slot_win[s]
                if k is None:
                    return
                wh, ws = slot_tile[s]
                # h = wh/(ws+eps) ; v = SU.state + b + h ; stats ; stash v
                den = wp.tile([P, 512], F32, tag="den")
                nc.vector.tensor_scalar_add(den[:], ws[:], 1e-6)
                nc.vector.reciprocal(den[:], den[:])
                hsl = wp.tile([P, 512], F32, tag="hsl")
                nc.vector.tensor_mul(hsl[:], wh[:], den[:])
                if kind == "node":
                    rhs_u = xT_own[:, k * 512:(k + 1) * 512]
                else:
                    rhs_u = wp.tile([P, 512], F32, tag="yupd")
                    nc.sync.dma_start(out=rhs_u[:], in_=dyT[:, k * 512:(k + 1) * 512])
                    rhs_u = rhs_u[:]
                pmu = pp.tile([P, 512], F32, tag="pm")
                nc.tensor.matmul(out=pmu[:], lhsT=W(pref + "su_W")[:], rhs=rhs_u,
                                 start=True, stop=True, **MM)
                v = wp.tile([P, 512], F32, tag="vup")
                nc.vector.tensor_scalar_add(v[:], pmu[:], W(pref + "su_b")[:])
                nc.vector.tensor_add(v[:], v[:], hsl[:])
                msk = wp.tile([P, 512], F32, tag="vmask")
                if kind == "node":
                    nc.sync.dma_start(out=msk[:], in_=nmask_bc[:, k * 512:(k + 1) * 512])
                else:
                    nc.sync.dma_start(out=msk[:], in_=emask_row_bc[:, k * 512:(k + 1) * 512])
                accum_stats(stat_a2, v[:], msk[:], P, k == 0)
                nc.sync.dma_start(out=dvb[:, k * 512:(k + 1) * 512], in_=v[:])
                slot_win[s] = None

            def open_slot(k):
                s = k % 2
                close_slot(s)
                slot_win[s] = k
                wh = ppw.tile([P, 512], F32, tag=f"pwh{s}")
                ws = ppw.tile([P, 512], F32, tag=f"pws{s}")
                slot_tile[s] = (wh, ws)

            _gp_cm = tc.tile_pool(name="g" + pref, bufs=2)
            gp = _gp_cm.__enter__()
            for q in range(0, n_tiles, 8):
                gt = gp.tile([P, 8 * 2 * H], F32, tag="gt")
                if kind == "node":
                    nc.gpsimd.dma_gather(
                        out_ap=gt[:].rearrange("p (j e) -> p j e", e=2 * H),
                        in_ap=dT1x[:],
                        idxs_ap=idx_sb[:, q * 8:q * 8 + 64],
                        num_idxs=1024, num_idxs_reg=1024, elem_size=2 * H)
                else:
                    for jj in range(8):
                        j = q + jj
                        nc.gpsimd.indirect_dma_start(
                            out=gt[:, jj * 2 * H:(jj + 1) * 2 * H],
                            out_offset=None, in_=dT1y[:],
                            in_offset=bass.IndirectOffsetOnAxis(
                                ap=idx_sb[:, j:j + 1], axis=0))
                # batched chunk loads: 1024 edges worth of dstrow / edge-gate /
                # (line) T2 blocks, issued on two HWDGE sequencers
                drc = gp.tile([P, 8 * P], F32, tag="drc")
                nc.sync.dma_start(out=drc[:], in_=dstrow_i[:, q * P:(q + 8) * P])
                egc = gp.tile([P, 8 * P], F32, tag="egc")
                nc.scalar.dma_start(out=egc[:], in_=edge_d[:, q * P:(q + 8) * P])
                if kind != "node":
                    blo, bhi = pl.t_chunk_rng[q // 8]
                    nbl = bhi - blo + 1
                    t2c = gp.tile([P, pl.t_max_nbl * P], F32, tag="t2c")
                    nc.sync.dma_start(
                        out=t2c[:, 0:nbl * P],
                        in_=dT2y[:].rearrange("(b w) f -> b w f", w=P)[blo:bhi + 1])
                mtc = gp.tile([P, 8 * P], F32, tag="mtc")
                for jj in range(8):
                    j = q + jj
                    if j >= n_tiles:
                        break
                    gA = gt[:, jj * 2 * H: jj * 2 * H + H]
                    gB = gt[:, jj * 2 * H + H: (jj + 1) * 2 * H]
                    pm = pp.tile([P, P], F32, tag="pm")
                    first = True
                    dr = drc[:, jj * P:(jj + 1) * P]
                    for b in blocks[j]:
                        tloc = wp.tile([P, P], F32, tag="tloc")
                        nc.vector.tensor_scalar(tloc[:], dr, float(b * P), None,
                                                ALU.subtract)
                        s_we = wp.tile([P, P], F32, tag="s_we")
                        nc.vector.tensor_tensor(
                            out=s_we[:], in0=iotaP[:].to_broadcast([P, P]),
                            in1=tloc[:], op=ALU.is_equal)
                        if kind == "node":
                            rhs_blk = T2x_strip[:, b * P:(b + 1) * P]
                        else:
                            rhs_blk = t2c[:, (b - blo) * P:(b - blo + 1) * P]
                        nc.tensor.matmul(out=pm[:], lhsT=s_we[:], rhs=rhs_blk,
                                         start=first, stop=False, **MM)
                        first = False
                    nc.tensor.matmul(out=pm[:], lhsT=egc[:, jj * P:(jj + 1) * P],
                                     rhs=W(pref + "eg_W")[:],
                                     start=first, stop=True, **MM)
                    m_sb = wp.tile([P, P], F32, tag="m_sb")
                    nc.vector.tensor_add(m_sb[:], pm[:], gA)
                    nc.vector.tensor_add(m_sb[:], m_sb[:], bm_bc[:])
                    sig = wp.tile([P, P], F32, tag="sig")
                    nc.scalar.activation(sig[:], m_sb[:], AF.Sigmoid)
                    bhs = wp.tile([P, P], F32, tag="bhs")
                    nc.vector.tensor_mul(bhs[:], gB, sig[:])
                    ptm = pp.tile([P, P], F32, tag="pm")
                    nc.tensor.transpose(out=ptm[:], in_=m_sb[:], identity=ident[:])
                    nc.scalar.activation(mtc[:, jj * P:(jj + 1) * P], ptm[:], AF.Copy)
                    for k in tile_wins[j]:
                        s = k % 2
                        if slot_win[s] != k:
                            open_slot(k)
                        wh, ws = slot_tile[s]
                        tl2 = wp.tile([P, 1], F32, tag="dcol")
                        nc.vector.tensor_scalar(tl2[:], dstcol_sb[:, j:j + 1],
                                                float(k * 512), None, ALU.subtract)
                        s_ew = wp.tile([P, 512], F32, tag="s_ew")
                        nc.vector.tensor_tensor(
                            out=s_ew[:], in0=tl2[:].to_broadcast([P, 512]),
                            in1=iota_t[:], op=ALU.is_equal)
                        nc.tensor.matmul(out=wh[:], lhsT=bhs[:], rhs=s_ew[:],
                                         start=(win_first[k] == j),
                                         stop=(win_last[k] == j), **MM)
                        nc.tensor.matmul(out=ws[:], lhsT=sig[:], rhs=s_ew[:],
                                         start=(win_first[k] == j),
                                         stop=(win_last[k] == j), **MM)
                nc.scalar.dma_start(out=dmT[:, q * P:(q + 8) * P], in_=mtc[:])
                mrow = gp.tile([P, 8 * P], F32, tag="mrow")
                nc.sync.dma_start(out=mrow[:], in_=mask_row_i[:, q * P:(q + 8) * P])
                accum_stats2(stat_a1, mtc[:], mrow[:], q == 0)
            for s in (0, 1):
                close_slot(s)
            _gp_cm.__exit__(None, None, None)

            do_ar([(stat_a1, P, 0), (stat_a2, P, 2)])
            nc.vector.tensor_copy(stat_a1[:], ar_out_sb[:, 0:2])
            nc.vector.tensor_copy(stat_a2[:], ar_out_sb[:, 2:4])

            # node-state update: silu(bn(v)) + residual
            finish_bn(stat_a2, count_n, W(pref + "bnn_g")[:], W(pref + "bnn_b")[:])
            for k in range(WIN // 512):
                v = wp.tile([P, 512], F32, tag="vup")
                nc.sync.dma_start(out=v[:], in_=dvb[:, k * 512:(k + 1) * 512])
                nc.scalar.activation(v[:], v[:], AF.Silu, bias=bi[:], scale=sc[:])
                if kind == "node":
                    nc.vector.tensor_add(xT_own[:, k * 512:(k + 1) * 512],
                                         xT_own[:, k * 512:(k + 1) * 512], v[:])
                else:
                    yo = wp.tile([P, 512], F32, tag="yupd")
                    nc.sync.dma_start(out=yo[:], in_=dyT[:, k * 512:(k + 1) * 512])
                    nc.vector.tensor_add(yo[:], yo[:], v[:])
                    nc.sync.dma_start(out=dyT[:, k * 512:(k + 1) * 512], in_=yo[:])

            # edge-state update from dmT
            finish_bn(stat_a1, count_e, W(pref + "bne_g")[:], W(pref + "bne_b")[:])
            for c in range(CAP // 512):
                mv = wp.tile([P, 512], F32, tag="mvt")
                nc.sync.dma_start(out=mv[:], in_=dmT[:, c * 512:(c + 1) * 512])
                nc.scalar.activation(mv[:], mv[:], AF.Silu, bias=bi[:], scale=sc[:])
                es = wp.tile([P, 512], F32, tag="est")
                nc.sync.dma_start(out=es[:], in_=edge_d[:, c * 512:(c + 1) * 512])
                nc.vector.tensor_add(es[:], es[:], mv[:])
                nc.sync.dma_start(out=edge_d[:, c * 512:(c + 1) * 512], in_=es[:])

            if kind == "node":
                # allgather x
                nc.sync.dma_start(out=dxag_in[:], in_=xT_own[:, 0:WCAP])
                nc.gpsimd.collective_compute(
                    "AllGather", ALU.bypass,
                    ins=[dxag_in[:]], outs=[dxag[:]], replica_groups=[CORE_IDS])
                dx3 = dxag[:].rearrange("(d p) w -> d p w", p=P)
                for d in range(NC):
                    wd = int(pl.W_d[d])
                    if wd > 0:
                        nc.sync.dma_start(
                            out=xT[:, int(pl.node_cut[d]):int(pl.node_cut[d]) + wd],
                            in_=dx3[d, :, 0:wd])

        for pref, kind in pl.layer_prefixes:
            egcn_layer(pref, kind)

        # ============================================================ readout
        gm3 = G_mat[:].rearrange("(c p) g -> c p g", p=P)
        ph = ppw.tile([P, G], F32, tag="pwh0")
        for c in range(N // P):
            gmc = wp.tile([P, G], F32, tag="den")
            nc.sync.dma_start(out=gmc[:], in_=gm3[c])
            ptx = pp.tile([P, P], F32, tag="pm")
            nc.tensor.transpose(out=ptx[:], in_=xT[:, c * P:(c + 1) * P], identity=ident[:])
            xrow = wp.tile([P, P], F32, tag="xrow")
            nc.scalar.activation(xrow[:], ptx[:], AF.Copy)
            nc.tensor.matmul(out=ph[:], lhsT=xrow[:], rhs=gmc[:],
                             start=(c == 0), stop=(c == N // P - 1), **MM)
        hg = pool.tile([P, G], F32, tag="hg")
        nc.scalar.activation(hg[:], ph[:], AF.Copy)
        pfc = ppw.tile([G, 1], F32, tag="pws0")
        nc.tensor.matmul(out=pfc[:], lhsT=hg[:], rhs=W("fc_W")[:], start=True, stop=True, **MM)
        og = wp.tile([G, 1], F32, tag="og")
        nc.vector.tensor_add(og[:], pfc[:], W("fc_b")[:G])
        nc.sync.dma_start(out=out_p[:], in_=og[:])

        _stk.close()

    nc.finalize()
    return nc


def _in_maps(pl):
    shared = {
        "afT": pl.afT, "G_mat": pl.G_mat, "iota512_bc": pl.iota512_bc,
        "iotaP_col": pl.iotaP_col, "iotaN_col": pl.iotaN_col,
        "e_centers": pl.e_centers, "a_centers": pl.a_centers,
    }
    for k, v in pl.params.items():
        shared["p_" + k] = v
    maps = []
    for d in range(NC):
        dv = pl.dev[d]
        m = dict(shared)
        m.update({
            "src_idx16": dv.src_idx16, "dst_l_col": dv.dst_l_col,
            "emask_col": dv.emask_col, "dst_row_bc": dv.dst_row_bc,
            "emask_row_bc": dv.emask_row_bc,
            "ts_idx32": dv.ts_idx32, "tdst_l_col": dv.tdst_l_col,
            "tmask_col": dv.tmask_col, "tdst_row_bc": dv.tdst_row_bc,
            "tmask_row_bc": dv.tmask_row_bc,
            "bond_bc": dv.bond_bc, "angle_bc": dv.angle_bc,
            "nmask_bc": dv.nmask_bc, "ncut_bc": dv.ncut_bc,
        })
        maps.append(m)
    return maps


def _np_tree(v):
    if isinstance(v, dict):
        return {k: _np_tree(x) for k, x in v.items()}
    if isinstance(v, (list, tuple)):
        return [_np_tree(x) for x in v]
    return np.asarray(v)


def kernel(atom_features, bondlength, angle_features, params,
           edge_src, edge_dst, t_src, t_dst, node2graph):
    atom_features = np.asarray(atom_features, np.float32)
    bondlength = np.asarray(bondlength, np.float32)
    angle_features = np.asarray(angle_features, np.float32)
    edge_src = np.asarray(edge_src)
    edge_dst = np.asarray(edge_dst)
    t_src = np.asarray(t_src)
    t_dst = np.asarray(t_dst)
    node2graph = np.asarray(node2graph)
    params = _np_tree(params)
    pl = build_plan(atom_features, bondlength, angle_features, params,
                    edge_src, edge_dst, t_src, t_dst, node2graph)
    ncb = build_program(pl)
    res = run_bass_kernel_spmd(
        ncb, _in_maps(pl), list(range(NC)),
        trace=os.environ.get("BASS_KERNEL_TRACE", "0") == "1")
    kernel.last_exec_time_ns = res.exec_time_ns
    out = np.asarray(res.results[0]["out"]).reshape(-1)[:pl.G]
    return out.astype(np.float32)


kernel.last_exec_time_ns = None
